# revision 21
# baseline (speedup 1.0000x reference)
"""Trainium2 Bass kernel for nn_CognitiveManifold (geodesic RK2 step).

8 NeuronCores, pure data parallel: 8192 tokens/core, full inputs in, full
outputs out. Analytic metric derivatives + one 8x8 SPD LDL^T solve per
token. v4: bf16 tensor-engine path (fp32 for the clip-sensitive R-channel
and the LDL/solve), two pipelined 4096-token chunks, batched LDL k0 from an
SBUF-staged metric, L factors in a separate pack (no per-k column copies),
copies/casts on the Scalar engine, Softplus/Sigmoid activations.

Per-chunk layouts (TC=4096 tokens, token_local = 32*p + q):
  A (tokens on partitions): [128, (q=32, feat)] fp32
  B (features on partitions, tokens on free):
    (d)-space  [64  = 8*q3+d,   (H=4, 128p)]    q = 8H + q3
    (j)-space  [128 = 16*q3+j,  (H=4, 128p)]
    (mn)-space [128 = 64*qs+mn, (Pl4|H4|128p)]  q = 8H + 2Pl + qs
"""

import numpy as np
from ml_dtypes import bfloat16

try:  # concourse ships with the container; ensure it's importable
    import concourse  # noqa: F401
except ImportError:  # pragma: no cover
    import sys as _sys
    for _p in ("/opt/trn_rl_repo", "/root/.axon_site/_ro/trn_rl_repo"):
        if _p not in _sys.path:
            _sys.path.insert(0, _p)

LAST_EXEC_TIME_NS = None
LAST_TRACE = None
D = 8
NCORES = 8
NTOK = 8192
TC = 4096
NCHUNK = NTOK // TC
NP = 128
NQ = TC // NP      # 32


def _build_consts(L, W1, b1, W2, b2, Wr1, br1, Wr2, br2):
    L, W1, b1, W2, b2 = (np.asarray(a, np.float64) for a in (L, W1, b1, W2, b2))
    Wr1, br1, Wr2, br2 = (np.asarray(a, np.float64) for a in (Wr1, br1, Wr2, br2))
    G0 = L @ L.T + 1e-4 * np.eye(D)
    W2r = W2.reshape(16, D, D)
    W2sym = (0.5 * (W2r + np.swapaxes(W2r, 1, 2))).reshape(16, 64)
    b2r = b2.reshape(D, D)
    b2sym = (0.5 * (b2r + b2r.T)).reshape(64)
    W2sym2 = (W2r + np.swapaxes(W2r, 1, 2)).reshape(16, 64)
    Wdr0 = Wr1 * Wr2[:, 0][None, :]          # [r, j] = Wr1[r,j]*Wr2[j,0]

    def blockdiag(w, g):
        kin, mout = w.shape
        out = np.zeros((g * kin, g * mout), dtype=np.float64)
        for i in range(g):
            out[i * kin:(i + 1) * kin, i * mout:(i + 1) * mout] = w
        return out

    B = {}   # bf16 consts
    F = {}   # fp32 consts
    B["eye128b"] = np.eye(128)
    F["eye128"] = np.eye(128)
    B["bd_w1"] = blockdiag(W1, 8)                 # [64,128]
    F["bd_wr1"] = blockdiag(Wr1, 8)               # [64,64] fp32 (R-channel)
    B["bd_g0"] = blockdiag(G0, 8)                 # [64,64]
    for Pl in range(4):
        w = np.zeros((128, 128))
        w2 = np.zeros((128, 128))
        for qs in range(2):
            q3 = 2 * Pl + qs
            w[q3 * 16:(q3 + 1) * 16, qs * 64:(qs + 1) * 64] = W2sym
            w2[q3 * 16:(q3 + 1) * 16, qs * 64:(qs + 1) * 64] = W2sym2
        B[f"bd_w2sym_{Pl}"] = w
        B[f"bd_w2sym2_{Pl}"] = w2
    B["bd_w2q"] = blockdiag(0.1 * W2.T, 2)        # [128,32]
    sel = np.zeros((128, 16))
    for qs in range(2):
        for n in range(D):
            for r in range(D):
                sel[qs * 64 + n * D + r, qs * D + r] = 1.0
    B["selc"] = sel
    ones2 = np.zeros((128, 2))
    ones2[:64, 0] = 1.0
    ones2[64:, 1] = 1.0
    B["onesc"] = ones2
    ones8 = np.zeros((64, 8))
    for q3 in range(8):
        ones8[q3 * D:(q3 + 1) * D, q3] = 1.0
    B["ones8c"] = ones8
    B["w1tc"] = blockdiag(W1.T, 8)                # [128,64]
    B["wdr0c"] = blockdiag(Wdr0.T, 8)             # [64,64]
    wr2c = np.zeros((64, 8))
    for q3 in range(8):
        wr2c[q3 * D:(q3 + 1) * D, q3] = Wr2[:, 0]
    F["wr2c"] = wr2c                              # fp32 (R-channel)
    for Pl in range(4):
        rep1 = np.zeros((64, 128))
        rep2 = np.zeros((64, 128))
        for qs in range(2):
            q3 = 2 * Pl + qs
            for d in range(D):
                for r in range(D):
                    rep1[q3 * D + d, qs * 64 + d * D + r] = 1.0   # n = d
                    rep2[q3 * D + d, qs * 64 + r * D + d] = 1.0   # r = d
        B[f"rep1c_{Pl}"] = rep1
        B[f"rep2c_{Pl}"] = rep2
    F["b1c"] = np.tile(b1, 8)                     # [128]
    F["br1c"] = np.tile(br1, 8)                   # [64]
    F["b2symc"] = np.tile(b2sym, 2)               # [128]
    F["g0colB"] = np.tile((10.0 * G0).reshape(64), 2)  # [128]
    consts = {k: np.ascontiguousarray(v, dtype=bfloat16) for k, v in B.items()}
    consts.update({k: np.ascontiguousarray(v, dtype=np.float32)
                   for k, v in F.items()})
    return consts, float(br2[0])


CONST_DTYPES = dict(
    **{k: "bf16" for k in
       ["eye128b", "bd_w1", "bd_g0", "bd_w2q", "selc", "onesc", "ones8c",
        "w1tc", "wdr0c"]
       + [f"bd_w2sym_{p}" for p in range(4)]
       + [f"bd_w2sym2_{p}" for p in range(4)]
       + [f"rep1c_{p}" for p in range(4)]
       + [f"rep2c_{p}" for p in range(4)]},
    **{k: "f32" for k in
       ["eye128", "bd_wr1", "wr2c", "b1c", "br1c", "b2symc", "g0colB"]},
)

CONST_SHAPES = {
    "eye128b": (128, 128), "eye128": (128, 128),
    "bd_w1": (64, 128), "bd_wr1": (64, 64), "bd_g0": (64, 64),
    "bd_w2q": (128, 32),
    "selc": (128, 16), "onesc": (128, 2), "ones8c": (64, 8),
    "w1tc": (128, 64), "wdr0c": (64, 64), "wr2c": (64, 8),
    "b1c": (128,), "br1c": (64,), "b2symc": (128,), "g0colB": (128,),
    **{f"bd_w2sym_{p}": (128, 128) for p in range(4)},
    **{f"bd_w2sym2_{p}": (128, 128) for p in range(4)},
    **{f"rep1c_{p}": (64, 128) for p in range(4)},
    **{f"rep2c_{p}": (64, 128) for p in range(4)},
}


def _emit(nc, tc, ctx, dram, br2f):
    import concourse.mybir as mybir

    f32 = mybir.dt.float32
    bf = mybir.dt.bfloat16
    AF = mybir.ActivationFunctionType
    OP = mybir.AluOpType

    consts = ctx.enter_context(tc.tile_pool(name="consts", bufs=1))
    sbB = ctx.enter_context(tc.tile_pool(name="sbB", bufs=2))    # big bf16 B
    sbP = ctx.enter_context(tc.tile_pool(name="sbP", bufs=2))    # per-Pl bf16
    sbF = ctx.enter_context(tc.tile_pool(name="sbF", bufs=2))    # fwd B tiles
    sbA = ctx.enter_context(tc.tile_pool(name="sbA", bufs=2))    # A-layout f32
    wps = ctx.enter_context(tc.tile_pool(name="wps", bufs=1, space="PSUM"))
    v12 = ctx.enter_context(tc.tile_pool(name="v12", bufs=1, space="PSUM"))
    qgt = ctx.enter_context(tc.tile_pool(name="qgt", bufs=1, space="PSUM"))
    scps = ctx.enter_context(tc.tile_pool(name="scps", bufs=1, space="PSUM"))

    cs = {}
    for name, shape in CONST_SHAPES.items():
        dt = bf if CONST_DTYPES[name] == "bf16" else f32
        if len(shape) == 1:
            t = consts.tile([shape[0], 1], dt, name=name, tag=name)
            nc.sync.dma_start(out=t[:, :],
                              in_=dram[name].rearrange("(p one) -> p one", one=1))
        else:
            t = consts.tile(list(shape), dt, name=name, tag=name)
            nc.sync.dma_start(out=t[:, :], in_=dram[name][:, :])
        cs[name] = t
    identb = cs["eye128b"]
    ident32 = cs["eye128"]
    br2t = consts.tile([128, 1], f32, name="br2t")
    nc.vector.memset(br2t[:, :], br2f)
    br2h = consts.tile([128, 1], f32, name="br2h")
    nc.vector.memset(br2h[:, :], 0.5 * br2f)
    onet = consts.tile([128, 1], f32, name="onet")
    nc.vector.memset(onet[:, :], 1.0)
    # clip(softplus(u), .1, 10) indicator thresholds mapped back to u
    UC1 = float(np.log(np.expm1(0.1)))
    UC2 = float(np.log(np.expm1(10.0)))

    def dram_chunk(t, c):
        return t[c * TC:(c + 1) * TC, :].rearrange("(p q) d -> p (q d)", q=NQ)

    def transpose32(src, f32tag, bftag):
        """[128,256] A-(q32,d8) fp32 -> [64,512] f32 + bf16 SBUF copies."""
        o32 = sbF.tile([64, 512], f32, tag=f32tag, bufs=1)
        ob = sbF.tile([64, 512], bf, tag=bftag, bufs=1)
        pt = wps.tile([128, 512], f32, tag="S")
        for H in range(4):
            nc.tensor.matmul(pt[:64, H * 128:(H + 1) * 128],
                             src[:, H * 64:(H + 1) * 64],
                             ident32[:, :], is_transpose=True,
                             start=True, stop=True)
        nc.scalar.activation(o32[:, :], pt[:64, :], AF.Identity)
        nc.scalar.activation(ob[:, :], pt[:64, :], AF.Identity)
        return o32, ob

    def transpose_bf(src, tag):
        """[128,256] A-(q32,d8) bf16 -> [64,512] bf16 SBUF."""
        out = sbF.tile([64, 512], bf, tag=tag)
        pt = qgt.tile([128, 512], bf, tag="gt")
        for H in range(4):
            nc.tensor.matmul(pt[:64, H * 128:(H + 1) * 128],
                             src[:, H * 64:(H + 1) * 64],
                             identb[:, :], is_transpose=True,
                             start=True, stop=True)
        nc.scalar.activation(out[:, :], pt[:64, :], AF.Identity)
        return out

    def emit_call(xT32, xTb, vTb, vA, aA):
        """One christoffel+contraction; writes acceleration into aA [128,(q,8)]."""

        # ---------- Phase A: forward matmuls + activations ----------
        u_ps = wps.tile([128, 512], f32, tag="S")
        nc.tensor.matmul(u_ps[:, :], cs["bd_w1"][:, :], xTb[:, :],
                         start=True, stop=True)
        a1B = sbF.tile([128, 512], bf, tag="a1B")
        gpuB = sbF.tile([128, 512], bf, tag="gpuB")
        nc.scalar.activation(a1B[:, :], u_ps[:, :], AF.Gelu, bias=cs["b1c"][:, :])
        nc.scalar.activation(gpuB[:, :], u_ps[:, :], AF.Derivative_Gelu,
                             bias=cs["b1c"][:, :])
        s_ps = wps.tile([128, 512], f32, tag="bs")
        nc.tensor.matmul(s_ps[:64, :], cs["bd_wr1"][:, :], xT32[:, :],
                         start=True, stop=True)
        a2B = sbF.tile([64, 512], f32, tag="a2B")
        gpsB = sbF.tile([64, 512], bf, tag="gpsB")
        nc.scalar.activation(a2B[:, :], s_ps[:64, :], AF.Gelu,
                             bias=cs["br1c"][:, :])
        nc.scalar.activation(gpsB[:, :], s_ps[:64, :], AF.Derivative_Gelu,
                             bias=cs["br1c"][:, :])
        c_ps = wps.tile([128, 512], f32, tag="S")
        nc.tensor.matmul(c_ps[:, :], cs["bd_w1"][:, :], vTb[:, :],
                         start=True, stop=True)
        cgB = sbF.tile([128, 512], bf, tag="cgB")
        nc.vector.tensor_tensor(out=cgB[:, :], in0=c_ps[:, :], in1=gpuB[:, :],
                                op=OP.mult)
        gv_ps = wps.tile([128, 512], f32, tag="bs")
        nc.tensor.matmul(gv_ps[:64, :], cs["bd_g0"][:, :], vTb[:, :],
                         start=True, stop=True)
        m1B = sbF.tile([64, 512], bf, tag="m1B")
        nc.vector.tensor_tensor(out=m1B[:, :], in0=gv_ps[:64, :], in1=vTb[:, :],
                                op=OP.mult)

        # ---------- small packs into PSUM ----------
        # pack: [0:32)t | [32:64)QG | [64:96)QE | [128:384)dr0
        #       [384:640)T1E | [640:896)T2E       (P = 4H+Pl)
        pk = scps.tile([128, 1024], f32, tag="pack")
        for H in range(4):
            hsl = slice(H * 128, (H + 1) * 128)
            nc.tensor.matmul(pk[:, H * 8:(H + 1) * 8], a2B[:, hsl],
                             cs["wr2c"][:, :], start=True, stop=True)
            nc.tensor.matmul(pk[:, 32 + H * 8:32 + (H + 1) * 8], m1B[:, hsl],
                             cs["ones8c"][:, :], start=True, stop=True)
            nc.tensor.matmul(pk[:, 128 + H * 64:128 + (H + 1) * 64],
                             gpsB[:, hsl], cs["wdr0c"][:, :],
                             start=True, stop=True)

        # ---------- Phase B: (mn)-space stream, Pl-major ----------
        tanhSB = sbB.tile([128, 2048], bf, tag="tanhSB")
        tanhGB = sbB.tile([128, 2048], bf, tag="tanhGB")
        q_ps = qgt.tile([128, 512], f32, tag="qps")
        for Pl in range(4):
            psl = slice(Pl * 512, (Pl + 1) * 512)
            S_ps = wps.tile([128, 512], f32, tag="S")
            nc.tensor.matmul(S_ps[:, :], cs[f"bd_w2sym_{Pl}"][:, :], a1B[:, :],
                             start=True, stop=True)
            bs_ps = wps.tile([128, 512], f32, tag="bs")
            nc.tensor.matmul(bs_ps[:, :], cs[f"bd_w2sym2_{Pl}"][:, :], cgB[:, :],
                             start=True, stop=True)
            v1_ps = v12.tile([128, 512], f32, tag="v1")
            nc.tensor.matmul(v1_ps[:, :], cs[f"rep1c_{Pl}"][:, :], vTb[:, :],
                             start=True, stop=True)
            v2_ps = v12.tile([128, 512], f32, tag="v2")
            nc.tensor.matmul(v2_ps[:, :], cs[f"rep2c_{Pl}"][:, :], vTb[:, :],
                             start=True, stop=True)

            nc.scalar.activation(tanhSB[:, psl], S_ps[:, :], AF.Tanh,
                                 bias=cs["b2symc"][:, :])
            nc.scalar.activation(tanhGB[:, psl], tanhSB[:, psl], AF.Identity,
                                 bias=cs["g0colB"][:, :])
            sqB = sbP.tile([128, 512], bf, tag="sqB")
            nc.scalar.activation(sqB[:, :], tanhSB[:, psl], AF.Square)
            tanhpB = sbP.tile([128, 512], bf, tag="tanhpB")
            nc.vector.tensor_scalar(out=tanhpB[:, :], in0=sqB[:, :],
                                    scalar1=-1.0, scalar2=1.0,
                                    op0=OP.mult, op1=OP.add)
            vr1b = sbP.tile([128, 512], bf, tag="vr1b")
            nc.scalar.activation(vr1b[:, :], v1_ps[:, :], AF.Identity)
            vvTB = sbP.tile([128, 512], bf, tag="vvTB")
            nc.vector.tensor_tensor(out=vvTB[:, :], in0=v2_ps[:, :],
                                    in1=vr1b[:, :], op=OP.mult)
            wtB = sbP.tile([128, 512], bf, tag="wtB")
            nc.vector.tensor_tensor(out=wtB[:, :], in0=bs_ps[:, :],
                                    in1=tanhpB[:, :], op=OP.mult)
            t1preB = sbP.tile([128, 512], bf, tag="t1preB")
            nc.vector.tensor_tensor(out=t1preB[:, :], in0=wtB[:, :],
                                    in1=vr1b[:, :], op=OP.mult)
            ppB = sbP.tile([128, 512], bf, tag="ppB")
            nc.gpsimd.tensor_mul(ppB[:, :], tanhpB[:, :], vvTB[:, :])
            qqB = sbP.tile([128, 512], bf, tag="qqB")
            nc.gpsimd.tensor_mul(qqB[:, :], tanhSB[:, psl], vvTB[:, :])

            nc.tensor.matmul(q_ps[32 * Pl:32 * (Pl + 1), :],
                             cs["bd_w2q"][:, :], ppB[:, :],
                             start=True, stop=True, tile_position=(0, 32 * Pl))
            for H in range(4):
                P = 4 * H + Pl
                hpl = slice(H * 128, (H + 1) * 128)
                nc.tensor.matmul(pk[:, 384 + P * 16:384 + (P + 1) * 16],
                                 t1preB[:, hpl], cs["selc"][:, :],
                                 start=True, stop=True)
                nc.tensor.matmul(pk[:, 64 + P * 2:64 + (P + 1) * 2],
                                 qqB[:, hpl], cs["onesc"][:, :],
                                 start=True, stop=True)

        # ---------- q -> gpq -> T2E ----------
        gpqB = sbF.tile([128, 512], bf, tag="gpqB")
        nc.vector.tensor_tensor(out=gpqB[:, :], in0=q_ps[:, :], in1=gpuB[:, :],
                                op=OP.mult)
        for H in range(4):
            nc.tensor.matmul(pk[:, 640 + H * 64:640 + (H + 1) * 64],
                             gpqB[:, H * 128:(H + 1) * 128], cs["w1tc"][:, :],
                             start=True, stop=True)
        # stage the pack to SBUF, freeing the psum banks for the next call
        pkSB = sbA.tile([128, 1024], f32, tag="pkSB", name="pkSB")
        nc.scalar.activation(pkSB[:, :], pk[:, :], AF.Identity)
        t_v = pkSB[:, 0:32]
        qg_v = pkSB[:, 32:64]
        qe_v = pkSB[:, 64:96]
        dr0_v = pkSB[:, 128:384]
        t1e_v = pkSB[:, 384:640]
        t2e_v = pkSB[:, 640:896]

        # ---------- scalar channel (fp32) ----------
        def stile(tag):
            return sbA.tile([128, 32], f32, tag=tag, name=tag)
        rrawA, sigA, rA, rinvA, kapA, tmpA, uA, absA = (
            stile(t) for t in ["rrawA", "sigA", "rA", "rinvA", "kapA", "tmpA",
                               "uA", "absA"])
        # u = t + br2; softplus(u) = ln(exp(-|u|) + 1) + relu(u)
        nc.scalar.activation(uA[:, :], t_v, AF.Identity, bias=br2t[:, :])
        nc.scalar.activation(absA[:, :], t_v, AF.Abs, bias=br2t[:, :])
        nc.scalar.activation(absA[:, :], absA[:, :], AF.Exp, scale=-1.0)
        nc.scalar.activation(absA[:, :], absA[:, :], AF.Ln, bias=onet[:, :])
        nc.vector.tensor_scalar_max(rrawA[:, :], uA[:, :], 0.0)
        nc.vector.tensor_add(rrawA[:, :], rrawA[:, :], absA[:, :])
        # sigmoid(u) = 0.5 + 0.5*tanh(u/2)
        nc.scalar.activation(sigA[:, :], t_v, AF.Tanh, scale=0.5,
                             bias=br2h[:, :])
        nc.vector.tensor_scalar(out=sigA[:, :], in0=sigA[:, :], scalar1=0.5,
                                scalar2=0.5, op0=OP.mult, op1=OP.add)
        nc.vector.tensor_scalar_max(rA[:, :], rrawA[:, :], 0.1)
        nc.vector.tensor_scalar_min(rA[:, :], rA[:, :], 10.0)
        nc.vector.reciprocal(rinvA[:, :], rA[:, :])
        # clip-derivative indicator on u directly (exact thresholds)
        nc.vector.tensor_scalar(out=kapA[:, :], in0=uA[:, :], scalar1=UC1,
                                scalar2=None, op0=OP.is_gt)
        nc.vector.tensor_scalar(out=tmpA[:, :], in0=uA[:, :], scalar1=UC2,
                                scalar2=None, op0=OP.is_lt)
        nc.vector.tensor_mul(kapA[:, :], kapA[:, :], tmpA[:, :])
        nc.vector.tensor_mul(kapA[:, :], kapA[:, :], sigA[:, :])

        # ---------- Phase C: gt transposes -> gtSB, batched LDL k=0 ----------
        gtSB = sbB.tile([128, 2048], bf, tag="gtSB")
        for H in range(4):
            gt_ps = qgt.tile([128, 512], bf, tag="gt")
            for Pl in range(4):
                nc.tensor.matmul(
                    gt_ps[:, Pl * 128:(Pl + 1) * 128],
                    tanhGB[:, Pl * 512 + H * 128:Pl * 512 + (H + 1) * 128],
                    identb[:, :], is_transpose=True, start=True, stop=True)
            nc.scalar.activation(gtSB[:, H * 512:(H + 1) * 512], gt_ps[:, :],
                                 AF.Identity)

        gA = sbA.tile([128, 2048], f32, tag="gA", name="gA")
        Lp = sbA.tile([128, 2048], f32, tag="Lp", name="Lp")   # (k8, q32, i8)
        invdA = sbA.tile([128, 256], f32, tag="invdA")
        tscrA = sbA.tile([128, 1568], f32, tag="tscrA")        # (q32, 49)
        gAv = gA[:, :].rearrange("p (q i j) -> p q i j", i=8, j=8)
        Lpv = Lp[:, :].rearrange("p (k q i) -> p k q i", k=8, i=8)
        gtv = gtSB[:, :].rearrange("p (q i j) -> p q i j", i=8, j=8)
        tv = tscrA[:, :].rearrange("p (q i j) -> p q i j", i=7, j=7)
        nc.vector.reciprocal(invdA[:, 0:32], gtv[:, :, 0, 0])
        nc.vector.tensor_tensor(
            out=Lpv[:, 0, :, 1:8], in0=gtv[:, :, 1:8, 0],
            in1=invdA[:, 0:32, None].broadcast_to([128, 32, 7]), op=OP.mult)
        QD = 20
        nc.vector.tensor_tensor(
            out=tv[:, 0:QD, :, :],
            in0=Lpv[:, 0, 0:QD, 1:8, None].broadcast_to([128, QD, 7, 7]),
            in1=gtv[:, 0:QD, None, 1:8, 0].broadcast_to([128, QD, 7, 7]),
            op=OP.mult)
        nc.gpsimd.tensor_tensor(
            out=tv[:, QD:32, :, :],
            in0=Lpv[:, 0, QD:32, 1:8, None].broadcast_to([128, 32 - QD, 7, 7]),
            in1=gtv[:, QD:32, None, 1:8, 0].broadcast_to([128, 32 - QD, 7, 7]),
            op=OP.mult)
        nc.vector.tensor_tensor(
            out=gAv[:, 0:QD, 1:8, 1:8], in0=gtv[:, 0:QD, 1:8, 1:8],
            in1=tv[:, 0:QD, :, :], op=OP.subtract)
        nc.gpsimd.tensor_tensor(
            out=gAv[:, QD:32, 1:8, 1:8], in0=gtv[:, QD:32, 1:8, 1:8],
            in1=tv[:, QD:32, :, :], op=OP.subtract)

        # ---------- LDL k=1..7 (all 32 q at once; L into Lp) ----------
        for k in range(1, 7):
            m = 7 - k
            nc.vector.reciprocal(invdA[:, 32 * k:32 * (k + 1)], gAv[:, :, k, k])
            nc.vector.tensor_tensor(
                out=Lpv[:, k, :, k + 1:8], in0=gAv[:, :, k + 1:8, k],
                in1=invdA[:, 32 * k:32 * (k + 1), None].broadcast_to([128, 32, m]),
                op=OP.mult)
            nc.vector.tensor_tensor(
                out=tv[:, 0:QD, :m, :m],
                in0=Lpv[:, k, 0:QD, k + 1:8, None].broadcast_to([128, QD, m, m]),
                in1=gAv[:, 0:QD, None, k + 1:8, k].broadcast_to([128, QD, m, m]),
                op=OP.mult)
            nc.gpsimd.tensor_tensor(
                out=tv[:, QD:32, :m, :m],
                in0=Lpv[:, k, QD:32, k + 1:8, None]
                    .broadcast_to([128, 32 - QD, m, m]),
                in1=gAv[:, QD:32, None, k + 1:8, k]
                    .broadcast_to([128, 32 - QD, m, m]),
                op=OP.mult)
            nc.vector.tensor_tensor(
                out=gAv[:, 0:QD, k + 1:8, k + 1:8],
                in0=gAv[:, 0:QD, k + 1:8, k + 1:8],
                in1=tv[:, 0:QD, :m, :m], op=OP.subtract)
            nc.gpsimd.tensor_tensor(
                out=gAv[:, QD:32, k + 1:8, k + 1:8],
                in0=gAv[:, QD:32, k + 1:8, k + 1:8],
                in1=tv[:, QD:32, :m, :m], op=OP.subtract)
        nc.vector.reciprocal(invdA[:, 224:256], gAv[:, :, 7, 7])

        # ---------- Q, coefZ, z ----------
        qaA, czA, caA, dvA = (stile(t) for t in ["qaA", "czA", "caA", "dvA"])
        nc.vector.scalar_tensor_tensor(out=qaA[:, :], in0=qe_v, scalar=0.1,
                                       in1=qg_v, op0=OP.mult, op1=OP.add)
        nc.vector.tensor_mul(czA[:, :], qaA[:, :], kapA[:, :])
        nc.vector.tensor_mul(czA[:, :], czA[:, :], rinvA[:, :])
        dvmA = sbA.tile([128, 256], f32, tag="dvmA")
        nc.vector.tensor_mul(dvmA[:, :], dr0_v, vA[:, :])
        nc.vector.tensor_reduce(
            dvA[:, :], dvmA[:, :].rearrange("p (q r) -> p q r", r=8),
            axis=mybir.AxisListType.X, op=OP.add)
        nc.vector.scalar_tensor_tensor(out=caA[:, :], in0=kapA[:, :], scalar=2.0,
                                       in1=dvA[:, :], op0=OP.mult, op1=OP.mult)
        nc.vector.tensor_mul(caA[:, :], caA[:, :], rinvA[:, :])
        # z = 0.05*T1E - 0.5*T2E - cz*dr0
        t1s = sbA.tile([128, 256], f32, tag="t1s")
        zA = sbA.tile([128, 256], f32, tag="zA")
        nc.vector.tensor_tensor(
            out=t1s[:, :].rearrange("p (q r) -> p q r", r=8),
            in0=dr0_v.rearrange("p (q r) -> p q r", r=8),
            in1=czA[:, :, None].broadcast_to([128, 32, 8]),
            op=OP.mult)
        nc.vector.scalar_tensor_tensor(out=zA[:, :], in0=t2e_v, scalar=-0.5,
                                       in1=t1s[:, :], op0=OP.mult,
                                       op1=OP.subtract)
        nc.vector.scalar_tensor_tensor(out=zA[:, :], in0=t1e_v, scalar=0.05,
                                       in1=zA[:, :], op0=OP.mult, op1=OP.add)

        # ---------- solve L D L^T y = z ----------
        yv = zA[:, :].rearrange("p (q r) -> p q r", r=8)        # in-place y
        sv = sbA.tile([128, 224], f32, tag="solve_scr")
        svv = sv[:, :].rearrange("p (q i) -> p q i", i=7)
        for k in range(0, 7):
            m = 7 - k
            nc.vector.tensor_tensor(
                out=svv[:, :, :m], in0=Lpv[:, k, :, k + 1:8],
                in1=yv[:, :, k:k + 1].broadcast_to([128, 32, m]), op=OP.mult)
            nc.vector.tensor_tensor(
                out=yv[:, :, k + 1:8], in0=yv[:, :, k + 1:8],
                in1=svv[:, :, :m], op=OP.subtract)
        nc.vector.tensor_tensor(
            out=yv[:, :, :],
            in0=yv[:, :, :],
            in1=invdA[:, :].rearrange("p (k q) -> p q k", q=32),
            op=OP.mult)
        for k in range(7, 0, -1):
            nc.vector.tensor_tensor(
                out=svv[:, :, :k],
                in0=Lpv[:, 0:k, :, k].rearrange("p j q -> p q j"),
                in1=yv[:, :, k:k + 1].broadcast_to([128, 32, k]), op=OP.mult)
            nc.vector.tensor_tensor(
                out=yv[:, :, 0:k], in0=yv[:, :, 0:k],
                in1=svv[:, :, :k], op=OP.subtract)

        # ---------- a = -coefA*v - 10*y ----------
        t3 = sbA.tile([128, 256], f32, tag="t3")
        nc.vector.tensor_tensor(
            out=t3[:, :].rearrange("p (q r) -> p q r", r=8),
            in0=vA[:, :].rearrange("p (q r) -> p q r", r=8),
            in1=caA[:, :, None].broadcast_to([128, 32, 8]),
            op=OP.mult)
        nc.vector.scalar_tensor_tensor(out=aA[:, :], in0=zA[:, :], scalar=-10.0,
                                       in1=t3[:, :], op0=OP.mult,
                                       op1=OP.subtract)

    # ================= chunk loop =================
    # preload + transpose both chunks first so chunk 1 work can overlap
    # chunk 0's first call
    pre = []
    for c in range(NCHUNK):
        xA = sbA.tile([128, 256], f32, tag=f"xA{c}", bufs=1)
        vA = sbA.tile([128, 256], f32, tag=f"vA{c}", bufs=1)
        nc.sync.dma_start(out=xA[:, :], in_=dram_chunk(dram["x"], c))
        nc.sync.dma_start(out=vA[:, :], in_=dram_chunk(dram["v"], c))
        xT32, xTb = transpose32(xA, f"xT32{c}", f"xTb{c}")
        vT32, vTb = transpose32(vA, f"vT32{c}", f"vTb{c}")
        xmidT32 = sbF.tile([64, 512], f32, tag=f"xmidT32{c}", bufs=1)
        nc.vector.scalar_tensor_tensor(out=xmidT32[:, :], in0=vT32[:, :],
                                       scalar=0.05, in1=xT32[:, :],
                                       op0=OP.mult, op1=OP.add)
        xmidTb = sbF.tile([64, 512], bf, tag=f"xmidTb{c}", bufs=1)
        nc.scalar.activation(xmidTb[:, :], xmidT32[:, :], AF.Identity)
        pre.append((xA, vA, xT32, xTb, vTb, xmidT32, xmidTb))

    for c in range(NCHUNK):
        xA, vA, xT32, xTb, vTb, xmidT32, xmidTb = pre[c]
        aA1 = sbA.tile([128, 256], f32, tag="aA1")
        emit_call(xT32, xTb, vTb, vA, aA1)

        vmidA = sbA.tile([128, 256], f32, tag="vmidA")
        nc.vector.scalar_tensor_tensor(out=vmidA[:, :], in0=aA1[:, :],
                                       scalar=0.05, in1=vA[:, :],
                                       op0=OP.mult, op1=OP.add)
        xnewA = sbA.tile([128, 256], f32, tag="xnewA")
        nc.vector.scalar_tensor_tensor(out=xnewA[:, :], in0=vmidA[:, :],
                                       scalar=0.1, in1=xA[:, :],
                                       op0=OP.mult, op1=OP.add)
        nc.sync.dma_start(out=dram_chunk(dram["x_new"], c), in_=xnewA[:, :])

        vmidb = sbA.tile([128, 256], bf, tag="vmidb")
        nc.scalar.activation(vmidb[:, :], vmidA[:, :], AF.Identity)
        vmidTb = transpose_bf(vmidb, "vmidTb")

        aA2 = sbA.tile([128, 256], f32, tag="aA2")
        emit_call(xmidT32, xmidTb, vmidTb, vmidA, aA2)

        vnewA = sbA.tile([128, 256], f32, tag="vnewA")
        nc.vector.scalar_tensor_tensor(out=vnewA[:, :], in0=aA2[:, :],
                                       scalar=0.1, in1=vA[:, :],
                                       op0=OP.mult, op1=OP.add)
        nc.sync.dma_start(out=dram_chunk(dram["v_new"], c), in_=vnewA[:, :])


def _build_module(consts, br2f):
    import concourse.bacc as bacc
    import concourse.mybir as mybir
    import concourse.tile as tile
    from contextlib import ExitStack

    f32 = mybir.dt.float32
    bf = mybir.dt.bfloat16
    nc = bacc.Bacc("TRN2", target_bir_lowering=False, debug=False,
                   num_devices=NCORES)
    dram = {}
    dram["x"] = nc.dram_tensor("x", [NTOK, D], f32, kind="ExternalInput").ap()
    dram["v"] = nc.dram_tensor("v", [NTOK, D], f32, kind="ExternalInput").ap()
    for name, arr in consts.items():
        dt = bf if arr.dtype == bfloat16 else f32
        dram[name] = nc.dram_tensor(name, list(arr.shape), dt,
                                    kind="ExternalInput").ap()
    dram["x_new"] = nc.dram_tensor("x_new", [NTOK, D], f32,
                                   kind="ExternalOutput").ap()
    dram["v_new"] = nc.dram_tensor("v_new", [NTOK, D], f32,
                                   kind="ExternalOutput").ap()
    with tile.TileContext(nc) as tc:
        with ExitStack() as ctx:
            _emit(nc, tc, ctx, dram, br2f)
    nc.compile()
    return nc


def kernel(x, v, L, W1, b1, W2, b2, Wr1, br1, Wr2, br2):
    x = np.ascontiguousarray(np.asarray(x, dtype=np.float32))
    v = np.ascontiguousarray(np.asarray(v, dtype=np.float32))
    consts, br2f = _build_consts(L, W1, b1, W2, b2, Wr1, br1, Wr2, br2)
    nc = _build_module(consts, br2f)

    from concourse.bass_utils import run_bass_kernel_spmd
    in_maps = []
    for c in range(NCORES):
        m = {"x": np.ascontiguousarray(x[c]), "v": np.ascontiguousarray(v[c])}
        m.update(consts)
        in_maps.append(m)
    import os as _os
    trace = _os.environ.get("KERNEL_TRACE", "0") == "1"
    tmpdir = _os.environ.get("KERNEL_TRACE_DIR") or None
    res = run_bass_kernel_spmd(nc, in_maps, core_ids=list(range(NCORES)),
                               trace=trace, tmpdir=tmpdir)
    global LAST_EXEC_TIME_NS, LAST_TRACE
    LAST_EXEC_TIME_NS = res.exec_time_ns
    LAST_TRACE = res.instructions_and_trace
    x_new = np.stack([r["x_new"] for r in res.results]).astype(np.float32)
    v_new = np.stack([r["v_new"] for r in res.results]).astype(np.float32)
    return (x_new, v_new)


# revision 22
# speedup vs baseline: 1.0399x; 1.0399x over previous
"""Trainium2 Bass kernel for nn_CognitiveManifold (geodesic RK2 step).

8 NeuronCores, pure data parallel: 8192 tokens/core, full inputs in, full
outputs out. Analytic metric derivatives + one 8x8 SPD LDL^T solve per
token. v4: bf16 tensor-engine path (fp32 for the clip-sensitive R-channel
and the LDL/solve), two pipelined 4096-token chunks, batched LDL k0 from an
SBUF-staged metric, L factors in a separate pack (no per-k column copies),
copies/casts on the Scalar engine, Softplus/Sigmoid activations.

Per-chunk layouts (TC=4096 tokens, token_local = 32*p + q):
  A (tokens on partitions): [128, (q=32, feat)] fp32
  B (features on partitions, tokens on free):
    (d)-space  [64  = 8*q3+d,   (H=4, 128p)]    q = 8H + q3
    (j)-space  [128 = 16*q3+j,  (H=4, 128p)]
    (mn)-space [128 = 64*qs+mn, (Pl4|H4|128p)]  q = 8H + 2Pl + qs
"""

import numpy as np
from ml_dtypes import bfloat16

try:  # concourse ships with the container; ensure it's importable
    import concourse  # noqa: F401
except ImportError:  # pragma: no cover
    import sys as _sys
    for _p in ("/opt/trn_rl_repo", "/root/.axon_site/_ro/trn_rl_repo"):
        if _p not in _sys.path:
            _sys.path.insert(0, _p)

LAST_EXEC_TIME_NS = None
LAST_TRACE = None
D = 8
NCORES = 8
NTOK = 8192
TC = 4096
NCHUNK = NTOK // TC
NP = 128
NQ = TC // NP      # 32


def _build_consts(L, W1, b1, W2, b2, Wr1, br1, Wr2, br2):
    L, W1, b1, W2, b2 = (np.asarray(a, np.float64) for a in (L, W1, b1, W2, b2))
    Wr1, br1, Wr2, br2 = (np.asarray(a, np.float64) for a in (Wr1, br1, Wr2, br2))
    G0 = L @ L.T + 1e-4 * np.eye(D)
    W2r = W2.reshape(16, D, D)
    W2sym = (0.5 * (W2r + np.swapaxes(W2r, 1, 2))).reshape(16, 64)
    b2r = b2.reshape(D, D)
    b2sym = (0.5 * (b2r + b2r.T)).reshape(64)
    W2sym2 = (W2r + np.swapaxes(W2r, 1, 2)).reshape(16, 64)
    Wdr0 = Wr1 * Wr2[:, 0][None, :]          # [r, j] = Wr1[r,j]*Wr2[j,0]

    def blockdiag(w, g):
        kin, mout = w.shape
        out = np.zeros((g * kin, g * mout), dtype=np.float64)
        for i in range(g):
            out[i * kin:(i + 1) * kin, i * mout:(i + 1) * mout] = w
        return out

    B = {}   # bf16 consts
    F = {}   # fp32 consts
    B["eye128b"] = np.eye(128)
    F["eye128"] = np.eye(128)
    B["bd_w1"] = blockdiag(W1, 8)                 # [64,128]
    F["bd_wr1"] = blockdiag(Wr1, 8)               # [64,64] fp32 (R-channel)
    B["bd_g0"] = blockdiag(G0, 8)                 # [64,64]
    for Pl in range(4):
        w = np.zeros((128, 128))
        w2 = np.zeros((128, 128))
        for qs in range(2):
            q3 = 2 * Pl + qs
            w[q3 * 16:(q3 + 1) * 16, qs * 64:(qs + 1) * 64] = W2sym
            w2[q3 * 16:(q3 + 1) * 16, qs * 64:(qs + 1) * 64] = W2sym2
        B[f"bd_w2sym_{Pl}"] = w
        B[f"bd_w2sym2_{Pl}"] = w2
    B["bd_w2q"] = blockdiag(0.1 * W2.T, 2)        # [128,32]
    sel = np.zeros((128, 16))
    for qs in range(2):
        for n in range(D):
            for r in range(D):
                sel[qs * 64 + n * D + r, qs * D + r] = 1.0
    B["selc"] = sel
    ones2 = np.zeros((128, 2))
    ones2[:64, 0] = 1.0
    ones2[64:, 1] = 1.0
    B["onesc"] = ones2
    ones8 = np.zeros((64, 8))
    for q3 in range(8):
        ones8[q3 * D:(q3 + 1) * D, q3] = 1.0
    B["ones8c"] = ones8
    B["w1tc"] = blockdiag(W1.T, 8)                # [128,64]
    B["wdr0c"] = blockdiag(Wdr0.T, 8)             # [64,64]
    wr2c = np.zeros((64, 8))
    for q3 in range(8):
        wr2c[q3 * D:(q3 + 1) * D, q3] = Wr2[:, 0]
    F["wr2c"] = wr2c                              # fp32 (R-channel)
    for Pl in range(4):
        rep1 = np.zeros((64, 128))
        rep2 = np.zeros((64, 128))
        for qs in range(2):
            q3 = 2 * Pl + qs
            for d in range(D):
                for r in range(D):
                    rep1[q3 * D + d, qs * 64 + d * D + r] = 1.0   # n = d
                    rep2[q3 * D + d, qs * 64 + r * D + d] = 1.0   # r = d
        B[f"rep1c_{Pl}"] = rep1
        B[f"rep2c_{Pl}"] = rep2
    F["b1c"] = np.tile(b1, 8)                     # [128]
    F["br1c"] = np.tile(br1, 8)                   # [64]
    F["b2symc"] = np.tile(b2sym, 2)               # [128]
    F["g0colB"] = np.tile((10.0 * G0).reshape(64), 2)  # [128]
    consts = {k: np.ascontiguousarray(v, dtype=bfloat16) for k, v in B.items()}
    consts.update({k: np.ascontiguousarray(v, dtype=np.float32)
                   for k, v in F.items()})
    return consts, float(br2[0])


CONST_DTYPES = dict(
    **{k: "bf16" for k in
       ["eye128b", "bd_w1", "bd_g0", "bd_w2q", "selc", "onesc", "ones8c",
        "w1tc", "wdr0c"]
       + [f"bd_w2sym_{p}" for p in range(4)]
       + [f"bd_w2sym2_{p}" for p in range(4)]
       + [f"rep1c_{p}" for p in range(4)]
       + [f"rep2c_{p}" for p in range(4)]},
    **{k: "f32" for k in
       ["eye128", "bd_wr1", "wr2c", "b1c", "br1c", "b2symc", "g0colB"]},
)

CONST_SHAPES = {
    "eye128b": (128, 128), "eye128": (128, 128),
    "bd_w1": (64, 128), "bd_wr1": (64, 64), "bd_g0": (64, 64),
    "bd_w2q": (128, 32),
    "selc": (128, 16), "onesc": (128, 2), "ones8c": (64, 8),
    "w1tc": (128, 64), "wdr0c": (64, 64), "wr2c": (64, 8),
    "b1c": (128,), "br1c": (64,), "b2symc": (128,), "g0colB": (128,),
    **{f"bd_w2sym_{p}": (128, 128) for p in range(4)},
    **{f"bd_w2sym2_{p}": (128, 128) for p in range(4)},
    **{f"rep1c_{p}": (64, 128) for p in range(4)},
    **{f"rep2c_{p}": (64, 128) for p in range(4)},
}


def _emit(nc, tc, ctx, dram, br2f):
    import concourse.mybir as mybir

    f32 = mybir.dt.float32
    bf = mybir.dt.bfloat16
    AF = mybir.ActivationFunctionType
    OP = mybir.AluOpType

    consts = ctx.enter_context(tc.tile_pool(name="consts", bufs=1))
    sbB = ctx.enter_context(tc.tile_pool(name="sbB", bufs=2))    # big bf16 B
    sbP = ctx.enter_context(tc.tile_pool(name="sbP", bufs=2))    # per-Pl bf16
    sbF = ctx.enter_context(tc.tile_pool(name="sbF", bufs=2))    # fwd B tiles
    sbA = ctx.enter_context(tc.tile_pool(name="sbA", bufs=2))    # A-layout f32
    wps = ctx.enter_context(tc.tile_pool(name="wps", bufs=1, space="PSUM"))
    v12 = ctx.enter_context(tc.tile_pool(name="v12", bufs=1, space="PSUM"))
    qgt = ctx.enter_context(tc.tile_pool(name="qgt", bufs=1, space="PSUM"))
    scps = ctx.enter_context(tc.tile_pool(name="scps", bufs=1, space="PSUM"))

    cs = {}
    for name, shape in CONST_SHAPES.items():
        dt = bf if CONST_DTYPES[name] == "bf16" else f32
        if len(shape) == 1:
            t = consts.tile([shape[0], 1], dt, name=name, tag=name)
            nc.sync.dma_start(out=t[:, :],
                              in_=dram[name].rearrange("(p one) -> p one", one=1))
        else:
            t = consts.tile(list(shape), dt, name=name, tag=name)
            nc.sync.dma_start(out=t[:, :], in_=dram[name][:, :])
        cs[name] = t
    identb = cs["eye128b"]
    ident32 = cs["eye128"]
    br2t = consts.tile([128, 1], f32, name="br2t")
    nc.vector.memset(br2t[:, :], br2f)
    br2h = consts.tile([128, 1], f32, name="br2h")
    nc.vector.memset(br2h[:, :], 0.5 * br2f)
    onet = consts.tile([128, 1], f32, name="onet")
    nc.vector.memset(onet[:, :], 1.0)
    # clip(softplus(u), .1, 10) indicator thresholds mapped back to u
    UC1 = float(np.log(np.expm1(0.1)))
    UC2 = float(np.log(np.expm1(10.0)))

    def dram_chunk(t, c):
        return t[c * TC:(c + 1) * TC, :].rearrange("(p q) d -> p (q d)", q=NQ)

    def transpose32(src, f32tag, bftag):
        """[128,256] A-(q32,d8) fp32 -> [64,512] f32 + bf16 SBUF copies."""
        o32 = sbF.tile([64, 512], f32, tag=f32tag, bufs=1)
        ob = sbF.tile([64, 512], bf, tag=bftag, bufs=1)
        pt = wps.tile([128, 512], f32, tag="S")
        for H in range(4):
            nc.tensor.matmul(pt[:64, H * 128:(H + 1) * 128],
                             src[:, H * 64:(H + 1) * 64],
                             ident32[:, :], is_transpose=True,
                             start=True, stop=True)
        nc.scalar.activation(o32[:, :], pt[:64, :], AF.Identity)
        nc.scalar.activation(ob[:, :], pt[:64, :], AF.Identity)
        return o32, ob

    def transpose_bf(src, tag):
        """[128,256] A-(q32,d8) bf16 -> [64,512] bf16 SBUF."""
        out = sbF.tile([64, 512], bf, tag=tag)
        pt = qgt.tile([128, 512], bf, tag="gt")
        for H in range(4):
            nc.tensor.matmul(pt[:64, H * 128:(H + 1) * 128],
                             src[:, H * 64:(H + 1) * 64],
                             identb[:, :], is_transpose=True,
                             start=True, stop=True)
        nc.scalar.activation(out[:, :], pt[:64, :], AF.Identity)
        return out

    def emit_call(xT32, xTb, vTb, vA, aA):
        """One christoffel+contraction; writes acceleration into aA [128,(q,8)]."""

        # ---------- Phase A: forward matmuls + activations ----------
        u_ps = wps.tile([128, 512], f32, tag="S")
        nc.tensor.matmul(u_ps[:, :], cs["bd_w1"][:, :], xTb[:, :],
                         start=True, stop=True)
        a1B = sbF.tile([128, 512], bf, tag="a1B")
        gpuB = sbF.tile([128, 512], bf, tag="gpuB")
        nc.scalar.activation(a1B[:, :], u_ps[:, :], AF.Gelu, bias=cs["b1c"][:, :])
        nc.scalar.activation(gpuB[:, :], u_ps[:, :], AF.Derivative_Gelu,
                             bias=cs["b1c"][:, :])
        s_ps = wps.tile([128, 512], f32, tag="bs")
        nc.tensor.matmul(s_ps[:64, :], cs["bd_wr1"][:, :], xT32[:, :],
                         start=True, stop=True)
        a2B = sbF.tile([64, 512], f32, tag="a2B")
        gpsB = sbF.tile([64, 512], bf, tag="gpsB")
        nc.scalar.activation(a2B[:, :], s_ps[:64, :], AF.Gelu,
                             bias=cs["br1c"][:, :])
        nc.scalar.activation(gpsB[:, :], s_ps[:64, :], AF.Derivative_Gelu,
                             bias=cs["br1c"][:, :])
        c_ps = wps.tile([128, 512], f32, tag="S")
        nc.tensor.matmul(c_ps[:, :], cs["bd_w1"][:, :], vTb[:, :],
                         start=True, stop=True)
        cgB = sbF.tile([128, 512], bf, tag="cgB")
        nc.vector.tensor_tensor(out=cgB[:, :], in0=c_ps[:, :], in1=gpuB[:, :],
                                op=OP.mult)
        gv_ps = wps.tile([128, 512], f32, tag="bs")
        nc.tensor.matmul(gv_ps[:64, :], cs["bd_g0"][:, :], vTb[:, :],
                         start=True, stop=True)
        m1B = sbF.tile([64, 512], bf, tag="m1B")
        nc.vector.tensor_tensor(out=m1B[:, :], in0=gv_ps[:64, :], in1=vTb[:, :],
                                op=OP.mult)

        # ---------- small packs into PSUM ----------
        # pack: [0:32)t | [32:64)QG | [64:96)QE | [128:384)dr0
        #       [384:640)T1E | [640:896)T2E       (P = 4H+Pl)
        pk = scps.tile([128, 1024], f32, tag="pack")
        for H in range(4):
            hsl = slice(H * 128, (H + 1) * 128)
            nc.tensor.matmul(pk[:, H * 8:(H + 1) * 8], a2B[:, hsl],
                             cs["wr2c"][:, :], start=True, stop=True)
            nc.tensor.matmul(pk[:, 32 + H * 8:32 + (H + 1) * 8], m1B[:, hsl],
                             cs["ones8c"][:, :], start=True, stop=True)
            nc.tensor.matmul(pk[:, 128 + H * 64:128 + (H + 1) * 64],
                             gpsB[:, hsl], cs["wdr0c"][:, :],
                             start=True, stop=True)

        # ---------- Phase B: (mn)-space stream, Pl-major ----------
        tanhSB = sbB.tile([128, 2048], bf, tag="tanhSB")
        tanhGB = sbB.tile([128, 2048], bf, tag="tanhGB")
        q_ps = qgt.tile([128, 512], f32, tag="qps")
        for Pl in range(4):
            psl = slice(Pl * 512, (Pl + 1) * 512)
            S_ps = wps.tile([128, 512], f32, tag="S")
            nc.tensor.matmul(S_ps[:, :], cs[f"bd_w2sym_{Pl}"][:, :], a1B[:, :],
                             start=True, stop=True)
            bs_ps = wps.tile([128, 512], f32, tag="bs")
            nc.tensor.matmul(bs_ps[:, :], cs[f"bd_w2sym2_{Pl}"][:, :], cgB[:, :],
                             start=True, stop=True)
            v1_ps = v12.tile([128, 512], f32, tag="v1")
            nc.tensor.matmul(v1_ps[:, :], cs[f"rep1c_{Pl}"][:, :], vTb[:, :],
                             start=True, stop=True)
            v2_ps = v12.tile([128, 512], f32, tag="v2")
            nc.tensor.matmul(v2_ps[:, :], cs[f"rep2c_{Pl}"][:, :], vTb[:, :],
                             start=True, stop=True)

            nc.scalar.activation(tanhSB[:, psl], S_ps[:, :], AF.Tanh,
                                 bias=cs["b2symc"][:, :])
            nc.scalar.activation(tanhGB[:, psl], tanhSB[:, psl], AF.Identity,
                                 bias=cs["g0colB"][:, :])
            sqB = sbP.tile([128, 512], bf, tag="sqB")
            nc.scalar.activation(sqB[:, :], tanhSB[:, psl], AF.Square)
            tanhpB = sbP.tile([128, 512], bf, tag="tanhpB")
            nc.vector.tensor_scalar(out=tanhpB[:, :], in0=sqB[:, :],
                                    scalar1=-1.0, scalar2=1.0,
                                    op0=OP.mult, op1=OP.add)
            vr1b = sbP.tile([128, 512], bf, tag="vr1b")
            nc.scalar.activation(vr1b[:, :], v1_ps[:, :], AF.Identity)
            vvTB = sbP.tile([128, 512], bf, tag="vvTB")
            nc.vector.tensor_tensor(out=vvTB[:, :], in0=v2_ps[:, :],
                                    in1=vr1b[:, :], op=OP.mult)
            wtB = sbP.tile([128, 512], bf, tag="wtB")
            nc.vector.tensor_tensor(out=wtB[:, :], in0=bs_ps[:, :],
                                    in1=tanhpB[:, :], op=OP.mult)
            t1preB = sbP.tile([128, 512], bf, tag="t1preB")
            nc.vector.tensor_tensor(out=t1preB[:, :], in0=wtB[:, :],
                                    in1=vr1b[:, :], op=OP.mult)
            ppB = sbP.tile([128, 512], bf, tag="ppB")
            nc.gpsimd.tensor_mul(ppB[:, :], tanhpB[:, :], vvTB[:, :])
            qqB = sbP.tile([128, 512], bf, tag="qqB")
            nc.gpsimd.tensor_mul(qqB[:, :], tanhSB[:, psl], vvTB[:, :])

            nc.tensor.matmul(q_ps[32 * Pl:32 * (Pl + 1), :],
                             cs["bd_w2q"][:, :], ppB[:, :],
                             start=True, stop=True, tile_position=(0, 32 * Pl))
            for H in range(4):
                P = 4 * H + Pl
                hpl = slice(H * 128, (H + 1) * 128)
                nc.tensor.matmul(pk[:, 384 + P * 16:384 + (P + 1) * 16],
                                 t1preB[:, hpl], cs["selc"][:, :],
                                 start=True, stop=True)
                nc.tensor.matmul(pk[:, 64 + P * 2:64 + (P + 1) * 2],
                                 qqB[:, hpl], cs["onesc"][:, :],
                                 start=True, stop=True)

        # ---------- q -> gpq -> T2E ----------
        gpqB = sbF.tile([128, 512], bf, tag="gpqB")
        nc.vector.tensor_tensor(out=gpqB[:, :], in0=q_ps[:, :], in1=gpuB[:, :],
                                op=OP.mult)
        for H in range(4):
            nc.tensor.matmul(pk[:, 640 + H * 64:640 + (H + 1) * 64],
                             gpqB[:, H * 128:(H + 1) * 128], cs["w1tc"][:, :],
                             start=True, stop=True)
        # stage the pack to SBUF, freeing the psum banks for the next call
        pkSB = sbA.tile([128, 1024], f32, tag="pkSB", name="pkSB")
        nc.scalar.activation(pkSB[:, :], pk[:, :], AF.Identity)
        t_v = pkSB[:, 0:32]
        qg_v = pkSB[:, 32:64]
        qe_v = pkSB[:, 64:96]
        dr0_v = pkSB[:, 128:384]
        t1e_v = pkSB[:, 384:640]
        t2e_v = pkSB[:, 640:896]

        # ---------- scalar channel (fp32) ----------
        def stile(tag):
            return sbA.tile([128, 32], f32, tag=tag, name=tag)
        rrawA, sigA, rA, rinvA, kapA, tmpA, uA, absA = (
            stile(t) for t in ["rrawA", "sigA", "rA", "rinvA", "kapA", "tmpA",
                               "uA", "absA"])
        # u = t + br2; softplus(u) = ln(exp(-|u|) + 1) + relu(u)
        nc.scalar.activation(uA[:, :], t_v, AF.Identity, bias=br2t[:, :])
        nc.scalar.activation(absA[:, :], t_v, AF.Abs, bias=br2t[:, :])
        nc.scalar.activation(absA[:, :], absA[:, :], AF.Exp, scale=-1.0)
        nc.scalar.activation(absA[:, :], absA[:, :], AF.Ln, bias=onet[:, :])
        nc.vector.tensor_scalar_max(rrawA[:, :], uA[:, :], 0.0)
        nc.vector.tensor_add(rrawA[:, :], rrawA[:, :], absA[:, :])
        # sigmoid(u) = 0.5 + 0.5*tanh(u/2)
        nc.scalar.activation(sigA[:, :], t_v, AF.Tanh, scale=0.5,
                             bias=br2h[:, :])
        nc.vector.tensor_scalar(out=sigA[:, :], in0=sigA[:, :], scalar1=0.5,
                                scalar2=0.5, op0=OP.mult, op1=OP.add)
        nc.vector.tensor_scalar_max(rA[:, :], rrawA[:, :], 0.1)
        nc.vector.tensor_scalar_min(rA[:, :], rA[:, :], 10.0)
        nc.vector.reciprocal(rinvA[:, :], rA[:, :])
        # clip-derivative indicator on u directly (exact thresholds)
        nc.vector.tensor_scalar(out=kapA[:, :], in0=uA[:, :], scalar1=UC1,
                                scalar2=None, op0=OP.is_gt)
        nc.vector.tensor_scalar(out=tmpA[:, :], in0=uA[:, :], scalar1=UC2,
                                scalar2=None, op0=OP.is_lt)
        nc.vector.tensor_mul(kapA[:, :], kapA[:, :], tmpA[:, :])
        nc.vector.tensor_mul(kapA[:, :], kapA[:, :], sigA[:, :])

        # ---------- Phase C: gt transposes -> gtSB, batched LDL k=0 ----------
        gtSB = sbB.tile([128, 2048], bf, tag="gtSB")
        for H in range(4):
            gt_ps = qgt.tile([128, 512], bf, tag="gt")
            for Pl in range(4):
                nc.tensor.matmul(
                    gt_ps[:, Pl * 128:(Pl + 1) * 128],
                    tanhGB[:, Pl * 512 + H * 128:Pl * 512 + (H + 1) * 128],
                    identb[:, :], is_transpose=True, start=True, stop=True)
            nc.scalar.activation(gtSB[:, H * 512:(H + 1) * 512], gt_ps[:, :],
                                 AF.Identity)

        gA = sbA.tile([128, 2048], f32, tag="gA", name="gA")
        Lp = sbA.tile([128, 2048], f32, tag="Lp", name="Lp")   # (k8, q32, i8)
        invdA = sbA.tile([128, 256], f32, tag="invdA")
        tscrA = sbA.tile([128, 1568], f32, tag="tscrA")        # (q32, 49)
        gAv = gA[:, :].rearrange("p (q i j) -> p q i j", i=8, j=8)
        Lpv = Lp[:, :].rearrange("p (k q i) -> p k q i", k=8, i=8)
        gtv = gtSB[:, :].rearrange("p (q i j) -> p q i j", i=8, j=8)
        tv = tscrA[:, :].rearrange("p (q i j) -> p q i j", i=7, j=7)
        nc.vector.reciprocal(invdA[:, 0:32], gtv[:, :, 0, 0])
        nc.vector.tensor_tensor(
            out=Lpv[:, 0, :, 1:8], in0=gtv[:, :, 1:8, 0],
            in1=invdA[:, 0:32, None].broadcast_to([128, 32, 7]), op=OP.mult)
        nc.vector.tensor_tensor(
            out=tv[:, :, :, :],
            in0=Lpv[:, 0, :, 1:8, None].broadcast_to([128, 32, 7, 7]),
            in1=gtv[:, :, None, 1:8, 0].broadcast_to([128, 32, 7, 7]),
            op=OP.mult)
        nc.vector.tensor_tensor(
            out=gAv[:, :, 1:8, 1:8], in0=gtv[:, :, 1:8, 1:8],
            in1=tv[:, :, :, :], op=OP.subtract)

        # ---------- LDL k=1..7 (all 32 q at once; L into Lp) ----------
        for k in range(1, 7):
            m = 7 - k
            nc.vector.reciprocal(invdA[:, 32 * k:32 * (k + 1)], gAv[:, :, k, k])
            nc.vector.tensor_tensor(
                out=Lpv[:, k, :, k + 1:8], in0=gAv[:, :, k + 1:8, k],
                in1=invdA[:, 32 * k:32 * (k + 1), None].broadcast_to([128, 32, m]),
                op=OP.mult)
            nc.vector.tensor_tensor(
                out=tv[:, :, :m, :m],
                in0=Lpv[:, k, :, k + 1:8, None].broadcast_to([128, 32, m, m]),
                in1=gAv[:, :, None, k + 1:8, k].broadcast_to([128, 32, m, m]),
                op=OP.mult)
            nc.vector.tensor_tensor(
                out=gAv[:, :, k + 1:8, k + 1:8], in0=gAv[:, :, k + 1:8, k + 1:8],
                in1=tv[:, :, :m, :m], op=OP.subtract)
        nc.vector.reciprocal(invdA[:, 224:256], gAv[:, :, 7, 7])

        # ---------- Q, coefZ, z ----------
        qaA, czA, caA, dvA = (stile(t) for t in ["qaA", "czA", "caA", "dvA"])
        nc.vector.scalar_tensor_tensor(out=qaA[:, :], in0=qe_v, scalar=0.1,
                                       in1=qg_v, op0=OP.mult, op1=OP.add)
        nc.vector.tensor_mul(czA[:, :], qaA[:, :], kapA[:, :])
        nc.vector.tensor_mul(czA[:, :], czA[:, :], rinvA[:, :])
        dvmA = sbA.tile([128, 256], f32, tag="dvmA")
        nc.vector.tensor_mul(dvmA[:, :], dr0_v, vA[:, :])
        nc.vector.tensor_reduce(
            dvA[:, :], dvmA[:, :].rearrange("p (q r) -> p q r", r=8),
            axis=mybir.AxisListType.X, op=OP.add)
        nc.vector.scalar_tensor_tensor(out=caA[:, :], in0=kapA[:, :], scalar=2.0,
                                       in1=dvA[:, :], op0=OP.mult, op1=OP.mult)
        nc.vector.tensor_mul(caA[:, :], caA[:, :], rinvA[:, :])
        # z = 0.05*T1E - 0.5*T2E - cz*dr0
        t1s = sbA.tile([128, 256], f32, tag="t1s")
        zA = sbA.tile([128, 256], f32, tag="zA")
        nc.vector.tensor_tensor(
            out=t1s[:, :].rearrange("p (q r) -> p q r", r=8),
            in0=dr0_v.rearrange("p (q r) -> p q r", r=8),
            in1=czA[:, :, None].broadcast_to([128, 32, 8]),
            op=OP.mult)
        nc.vector.scalar_tensor_tensor(out=zA[:, :], in0=t2e_v, scalar=-0.5,
                                       in1=t1s[:, :], op0=OP.mult,
                                       op1=OP.subtract)
        nc.vector.scalar_tensor_tensor(out=zA[:, :], in0=t1e_v, scalar=0.05,
                                       in1=zA[:, :], op0=OP.mult, op1=OP.add)

        # ---------- solve L D L^T y = z ----------
        yv = zA[:, :].rearrange("p (q r) -> p q r", r=8)        # in-place y
        sv = sbA.tile([128, 224], f32, tag="solve_scr")
        svv = sv[:, :].rearrange("p (q i) -> p q i", i=7)
        for k in range(0, 7):
            m = 7 - k
            nc.vector.tensor_tensor(
                out=svv[:, :, :m], in0=Lpv[:, k, :, k + 1:8],
                in1=yv[:, :, k:k + 1].broadcast_to([128, 32, m]), op=OP.mult)
            nc.vector.tensor_tensor(
                out=yv[:, :, k + 1:8], in0=yv[:, :, k + 1:8],
                in1=svv[:, :, :m], op=OP.subtract)
        nc.vector.tensor_tensor(
            out=yv[:, :, :],
            in0=yv[:, :, :],
            in1=invdA[:, :].rearrange("p (k q) -> p q k", q=32),
            op=OP.mult)
        for k in range(7, 0, -1):
            nc.vector.tensor_tensor(
                out=svv[:, :, :k],
                in0=Lpv[:, 0:k, :, k].rearrange("p j q -> p q j"),
                in1=yv[:, :, k:k + 1].broadcast_to([128, 32, k]), op=OP.mult)
            nc.vector.tensor_tensor(
                out=yv[:, :, 0:k], in0=yv[:, :, 0:k],
                in1=svv[:, :, :k], op=OP.subtract)

        # ---------- a = -coefA*v - 10*y ----------
        t3 = sbA.tile([128, 256], f32, tag="t3")
        nc.vector.tensor_tensor(
            out=t3[:, :].rearrange("p (q r) -> p q r", r=8),
            in0=vA[:, :].rearrange("p (q r) -> p q r", r=8),
            in1=caA[:, :, None].broadcast_to([128, 32, 8]),
            op=OP.mult)
        nc.vector.scalar_tensor_tensor(out=aA[:, :], in0=zA[:, :], scalar=-10.0,
                                       in1=t3[:, :], op0=OP.mult,
                                       op1=OP.subtract)

    # ================= chunk loop =================
    # preload + transpose both chunks first so chunk 1 work can overlap
    # chunk 0's first call
    pre = []
    for c in range(NCHUNK):
        xA = sbA.tile([128, 256], f32, tag=f"xA{c}", bufs=1)
        vA = sbA.tile([128, 256], f32, tag=f"vA{c}", bufs=1)
        nc.sync.dma_start(out=xA[:, :], in_=dram_chunk(dram["x"], c))
        nc.sync.dma_start(out=vA[:, :], in_=dram_chunk(dram["v"], c))
        xT32, xTb = transpose32(xA, f"xT32{c}", f"xTb{c}")
        vT32, vTb = transpose32(vA, f"vT32{c}", f"vTb{c}")
        xmidT32 = sbF.tile([64, 512], f32, tag=f"xmidT32{c}", bufs=1)
        nc.vector.scalar_tensor_tensor(out=xmidT32[:, :], in0=vT32[:, :],
                                       scalar=0.05, in1=xT32[:, :],
                                       op0=OP.mult, op1=OP.add)
        xmidTb = sbF.tile([64, 512], bf, tag=f"xmidTb{c}", bufs=1)
        nc.scalar.activation(xmidTb[:, :], xmidT32[:, :], AF.Identity)
        pre.append((xA, vA, xT32, xTb, vTb, xmidT32, xmidTb))

    for c in range(NCHUNK):
        xA, vA, xT32, xTb, vTb, xmidT32, xmidTb = pre[c]
        aA1 = sbA.tile([128, 256], f32, tag="aA1")
        emit_call(xT32, xTb, vTb, vA, aA1)

        vmidA = sbA.tile([128, 256], f32, tag="vmidA")
        nc.vector.scalar_tensor_tensor(out=vmidA[:, :], in0=aA1[:, :],
                                       scalar=0.05, in1=vA[:, :],
                                       op0=OP.mult, op1=OP.add)
        xnewA = sbA.tile([128, 256], f32, tag="xnewA")
        nc.vector.scalar_tensor_tensor(out=xnewA[:, :], in0=vmidA[:, :],
                                       scalar=0.1, in1=xA[:, :],
                                       op0=OP.mult, op1=OP.add)
        nc.sync.dma_start(out=dram_chunk(dram["x_new"], c), in_=xnewA[:, :])

        vmidb = sbA.tile([128, 256], bf, tag="vmidb")
        nc.scalar.activation(vmidb[:, :], vmidA[:, :], AF.Identity)
        vmidTb = transpose_bf(vmidb, "vmidTb")

        aA2 = sbA.tile([128, 256], f32, tag="aA2")
        emit_call(xmidT32, xmidTb, vmidTb, vmidA, aA2)

        vnewA = sbA.tile([128, 256], f32, tag="vnewA")
        nc.vector.scalar_tensor_tensor(out=vnewA[:, :], in0=aA2[:, :],
                                       scalar=0.1, in1=vA[:, :],
                                       op0=OP.mult, op1=OP.add)
        nc.sync.dma_start(out=dram_chunk(dram["v_new"], c), in_=vnewA[:, :])


def _build_module(consts, br2f):
    import concourse.bacc as bacc
    import concourse.mybir as mybir
    import concourse.tile as tile
    from contextlib import ExitStack

    f32 = mybir.dt.float32
    bf = mybir.dt.bfloat16
    nc = bacc.Bacc("TRN2", target_bir_lowering=False, debug=False,
                   num_devices=NCORES)
    dram = {}
    dram["x"] = nc.dram_tensor("x", [NTOK, D], f32, kind="ExternalInput").ap()
    dram["v"] = nc.dram_tensor("v", [NTOK, D], f32, kind="ExternalInput").ap()
    for name, arr in consts.items():
        dt = bf if arr.dtype == bfloat16 else f32
        dram[name] = nc.dram_tensor(name, list(arr.shape), dt,
                                    kind="ExternalInput").ap()
    dram["x_new"] = nc.dram_tensor("x_new", [NTOK, D], f32,
                                   kind="ExternalOutput").ap()
    dram["v_new"] = nc.dram_tensor("v_new", [NTOK, D], f32,
                                   kind="ExternalOutput").ap()
    with tile.TileContext(nc) as tc:
        with ExitStack() as ctx:
            _emit(nc, tc, ctx, dram, br2f)
    nc.compile()
    return nc


def kernel(x, v, L, W1, b1, W2, b2, Wr1, br1, Wr2, br2):
    x = np.ascontiguousarray(np.asarray(x, dtype=np.float32))
    v = np.ascontiguousarray(np.asarray(v, dtype=np.float32))
    consts, br2f = _build_consts(L, W1, b1, W2, b2, Wr1, br1, Wr2, br2)
    nc = _build_module(consts, br2f)

    from concourse.bass_utils import run_bass_kernel_spmd
    in_maps = []
    for c in range(NCORES):
        m = {"x": np.ascontiguousarray(x[c]), "v": np.ascontiguousarray(v[c])}
        m.update(consts)
        in_maps.append(m)
    import os as _os
    trace = _os.environ.get("KERNEL_TRACE", "0") == "1"
    tmpdir = _os.environ.get("KERNEL_TRACE_DIR") or None
    res = run_bass_kernel_spmd(nc, in_maps, core_ids=list(range(NCORES)),
                               trace=trace, tmpdir=tmpdir)
    global LAST_EXEC_TIME_NS, LAST_TRACE
    LAST_EXEC_TIME_NS = res.exec_time_ns
    LAST_TRACE = res.instructions_and_trace
    x_new = np.stack([r["x_new"] for r in res.results]).astype(np.float32)
    v_new = np.stack([r["v_new"] for r in res.results]).astype(np.float32)
    return (x_new, v_new)
